# revision 13
# baseline (speedup 1.0000x reference)
"""ComplexBatchNorm2d (Trabelsi-style complex whitening BN) on 8 trn2 NeuronCores.

Sharding: over channels C (8 channels per core). Each channel's batch statistics
are computed entirely on one core, so no collectives are needed.

All HBM traffic is bf16 (the 2e-2 rel-err gate leaves ~50x headroom over bf16
quantization noise): the host downcasts inputs, the device writes bf16 planar
[re|im] outputs, and the host de-interleaves + upcasts. This halves the DMA
bytes vs fp32, which was the original bottleneck (79% DMA busy).

Channel data is PLANAR in SBUF ([X(4096) | Y(4096) | ones(2)] per channel) so
every whiten op is flat and contiguous -> DVE/ACT run in their 2x packed
16-bit modes (the interleaved-chunk layout forced 1x + per-row AP overhead).

Per-core device kernel (Bass/Tile), channel-pipelined in 2 groups of 4:
  stats:  2x2 Gram via TensorE bf16 matmuls in two flat passes per channel
          (BIR requires single-free-dim matmul operands): pass 1 loads each
          X_J 128-col chunk stationary and streams X_J (-> gXX), Y_J (-> gXY)
          and the ones column (-> X sums); pass 2 loads Y_J and streams Y_J
          (-> gYY) and ones (-> Y sums). Diagonals extracted with an
          eye128-masked TT-mult + reduce; a ones-matmul folds partitions.
  2x2:    closed-form (V + eps I)^{-1/2} batched over the 4 channels of a
          group ([P,4]-wide ops), folded with gamma/beta into
          y_re = G00*xr + G01*xi + BR (same for im); the 24 coefficients are
          broadcast to all partitions via one DRAM bounce per group.
  whiten: ScalarE (ACT) computes t = G*x + B for both components, VectorE adds
          the cross terms writing packed bf16 planar [re | im] halves; one
          contiguous 2 MB DMA out per channel.
Groups overlap: group B's loads/grams (DMA/PE) run while group A whitens
(ACT/DVE), keeping the DMA engines saturated.

Host side: slices/permutes inputs per core to planar bf16, gathers per-core
bf16 outputs and permutes back to (B, C, H, W, 2) f32.
"""

import numpy as np
import ml_dtypes

BF16 = ml_dtypes.bfloat16

# Problem geometry (hardcoded per contract).
B, C, H, W = 32, 64, 128, 128
NCORES = 8
CLOC = C // NCORES          # channels per core = 8
P = 128                     # SBUF partitions
N = B * H * W               # samples per channel = 524288
F = N // P                  # free columns per channel = 4096
CHUNK = 128                 # data columns per gram chunk (full PE width)
NCHUNK = F // CHUNK         # 32 chunks per channel
XYW = 2 * F + 2             # planar [X(F) | Y(F) | ones(2)] per channel
YW = 2 * F                  # 8192 output cols per channel: [re(F) | im(F)]
GRP = 4                     # channels per assembly group
NG = CLOC // GRP            # 2 groups
EPS = 1e-5

_CACHE = {}
_TRACE = False   # test.py sets this to capture NTFF profile / HW exec time
LAST = {}        # kernel() stores exec_time_ns etc. here

# tuning knobs (module-level so the bench harness can sweep them)
XY_BUFS = 5      # channel-data tiles in flight (each 16 KiB/partition)


def _build_nc():
    import concourse.bacc as bacc
    import concourse.mybir as mybir
    from concourse.tile import TileContext

    f32 = mybir.dt.float32
    bf16 = mybir.dt.bfloat16
    Alu = mybir.AluOpType
    Act = mybir.ActivationFunctionType
    Axis = mybir.AxisListType

    # Bacc (not raw Bass): Tile emits multi-wait sync_info that only the bacc
    # pipeline (nop/event-semaphore lowering) can legalize for walrus codegen.
    nc = bacc.Bacc("TRN2", target_bir_lowering=False)
    xy_d = nc.declare_dram_parameter("xy", [CLOC, P, XYW], bf16, isOutput=False)
    consts_d = nc.declare_dram_parameter("consts", [P, P], f32, isOutput=False)
    gb_d = nc.declare_dram_parameter("gb", [P, 48], f32, isOutput=False)
    y_d = nc.declare_dram_parameter("y", [CLOC, P, YW], bf16, isOutput=True)
    scratch_d = nc.dram_tensor("scratch", [NG, 6 * GRP], f32)

    V = nc.vector
    S = nc.scalar

    with TileContext(nc) as tc:
        with (
            tc.tile_pool(name="singles", bufs=1) as singles,
            tc.tile_pool(name="xyp", bufs=XY_BUFS) as xyp,
            tc.tile_pool(name="yp", bufs=2) as yp,
            tc.tile_pool(name="t1p", bufs=4) as t1p,
            tc.tile_pool(name="statp", bufs=2) as statp,
            tc.tile_pool(name="smallp", bufs=2) as smallp,
            tc.tile_pool(name="gramp", bufs=2, space="PSUM") as gramp,
            tc.tile_pool(name="spsum", bufs=1, space="PSUM") as spsump,
        ):
            consts = singles.tile([P, P], f32)
            nc.sync.dma_start(out=consts[:], in_=consts_d[:])
            gb = singles.tile([P, 48], f32)
            nc.sync.dma_start(out=gb[:], in_=gb_d[:])

            # DVE-staged eye128 so the masked-diag TT ops depend on at most
            # one cross-engine producer.
            ident = singles.tile([P, P], f32)
            V.tensor_copy(ident[:], consts[:])
            # f32 ones for the partition-fold matmul of the diag partials.
            ones_f32 = singles.tile([P, P], f32)
            V.memset(ones_f32[:], 1.0)

            for grp in range(NG):
                stats = statp.tile([P, 5 * GRP], f32, tag="stats")
                xts = []
                for ci in range(GRP):
                    c = grp * GRP + ci
                    # ---- load this channel's planar data (used twice) ----
                    xt = xyp.tile([P, XYW], bf16, tag="xy")
                    nc.sync.dma_start(out=xt[:], in_=xy_d[c])
                    xts.append(xt)
                    ones_col = xt[:, 2 * F:2 * F + 1]

                    # ---- gram passes: flat 128-col chunks ----
                    # gA[:, 0:128] = X^T X (diag -> sum xr^2)
                    # gB[:, 0:128] = X^T Y (diag -> sum xr*xi)
                    # gA[:, 128:256] = Y^T Y (diag -> sum xi^2)
                    # gS[:, 0] = per-col X sums; gS[:, 1] = per-col Y sums
                    gA = gramp.tile([P, 2 * P], f32, tag="gA")
                    gB = gramp.tile([P, P], f32, tag="gB")
                    gS = gramp.tile([P, 2], f32, tag="gS")
                    for j in range(NCHUNK):
                        xj = xt[:, j * CHUNK:(j + 1) * CHUNK]
                        yj = xt[:, F + j * CHUNK:F + (j + 1) * CHUNK]
                        st = (j == 0)
                        sp = (j == NCHUNK - 1)
                        nc.tensor.matmul(gA[:, 0:P], lhsT=xj, rhs=xj,
                                         start=st, stop=sp)
                        nc.tensor.matmul(gB[:, :], lhsT=xj, rhs=yj,
                                         start=st, stop=sp)
                        nc.tensor.matmul(gS[:, 0:1], lhsT=xj, rhs=ones_col,
                                         start=st, stop=sp)
                    for j in range(NCHUNK):
                        yj = xt[:, F + j * CHUNK:F + (j + 1) * CHUNK]
                        st = (j == 0)
                        sp = (j == NCHUNK - 1)
                        nc.tensor.matmul(gA[:, P:2 * P], lhsT=yj, rhs=yj,
                                         start=st, stop=sp)
                        nc.tensor.matmul(gS[:, 1:2], lhsT=yj, rhs=ones_col,
                                         start=st, stop=sp)

                    # ---- diag extraction into the group stats tile ----
                    junk = smallp.tile([P, P], f32, tag="junk")
                    V.tensor_mul(junk[:], gA[:, 0:P], ident[:])
                    V.tensor_reduce(out=stats[:, 0 * GRP + ci:0 * GRP + ci + 1],
                                    in_=junk[:], axis=Axis.X, op=Alu.add)
                    V.tensor_mul(junk[:], gB[:, :], ident[:])
                    V.tensor_reduce(out=stats[:, 1 * GRP + ci:1 * GRP + ci + 1],
                                    in_=junk[:], axis=Axis.X, op=Alu.add)
                    V.tensor_mul(junk[:], gA[:, P:2 * P], ident[:])
                    V.tensor_reduce(out=stats[:, 2 * GRP + ci:2 * GRP + ci + 1],
                                    in_=junk[:], axis=Axis.X, op=Alu.add)
                    S.copy(stats[:, 3 * GRP + ci:3 * GRP + ci + 1], gS[:, 0:1])
                    S.copy(stats[:, 4 * GRP + ci:4 * GRP + ci + 1], gS[:, 1:2])

                # partition fold: all 128 output rows hold all 5*GRP sums
                s_ps = spsump.tile([P, 5 * GRP], f32, tag="sps")
                nc.tensor.matmul(s_ps[:, :], lhsT=ones_f32[:], rhs=stats[:],
                                 start=True, stop=True)
                s_sb = smallp.tile([P, 5 * GRP], f32, tag="ssb")
                V.tensor_copy(s_sb[:], s_ps[:, :])

                # ---- 2x2 assembly, batched over the group's GRP channels,
                #      replicated across partitions ----
                def qs(q, t=None):
                    t = s_sb if t is None else t
                    return t[:, q * GRP:(q + 1) * GRP]

                SXX, SXY, SYY = qs(0), qs(1), qs(2)
                SR, SI = qs(3), qs(4)
                tmp = smallp.tile([P, 16 * GRP], f32, tag="tmp")

                def ts(i, tmp=tmp):
                    return tmp[:, i * GRP:(i + 1) * GRP]

                rN = 1.0 / N
                rN1 = 1.0 / (N - 1)
                MR, MI, u = ts(0), ts(1), ts(2)
                a, bb, cc = ts(3), ts(4), ts(5)
                V.tensor_scalar_mul(MR, SR, rN)
                V.tensor_scalar_mul(MI, SI, rN)
                # a=(Sxx-Sx*mr)/(N-1)+eps; b=(Sxy-Sx*mi)/(N-1);
                # c=(Syy-Sy*mi)/(N-1)+eps
                V.tensor_mul(u, SR, MR)
                V.tensor_sub(a, SXX, u)
                V.tensor_scalar(out=a, in0=a, scalar1=rN1, scalar2=EPS,
                                op0=Alu.mult, op1=Alu.add)
                V.tensor_mul(u, SR, MI)
                V.tensor_sub(bb, SXY, u)
                V.tensor_scalar_mul(bb, bb, rN1)
                V.tensor_mul(u, SI, MI)
                V.tensor_sub(cc, SYY, u)
                V.tensor_scalar(out=cc, in0=cc, scalar1=rN1, scalar2=EPS,
                                op0=Alu.mult, op1=Alu.add)
                # (M)^{-1/2} for M=[[a,b],[b,c]]: s=sqrt(ac-b^2);
                # t=sqrt(a+c+2s); W=[[c+s,-b],[-b,a+s]]/(s*t)
                det, s_, tr, st_, inv = ts(6), ts(7), ts(8), ts(9), ts(10)
                V.tensor_mul(det, a, cc)
                V.tensor_mul(u, bb, bb)
                V.tensor_sub(det, det, u)
                nc.scalar.sqrt(s_, det)
                V.tensor_add(u, a, cc)
                V.tensor_scalar_mul(tr, s_, 2.0)
                V.tensor_add(tr, tr, u)
                nc.scalar.sqrt(tr, tr)
                V.tensor_mul(st_, s_, tr)
                V.reciprocal(inv, st_)
                w00, w01, w11, q = ts(11), ts(12), ts(13), ts(14)
                V.tensor_add(w00, cc, s_)
                V.tensor_mul(w00, w00, inv)
                V.scalar_tensor_tensor(out=w01, in0=bb, scalar=-1.0, in1=inv,
                                       op0=Alu.mult, op1=Alu.mult)
                V.tensor_add(w11, a, s_)
                V.tensor_mul(w11, w11, inv)
                # G = gamma @ W ; B' = beta - G @ mean  (gb is quantity-major
                # over 8 channels; this group's 4 are contiguous)
                g00 = gb[:, 0 * 8 + grp * GRP: 0 * 8 + grp * GRP + GRP]
                g01 = gb[:, 1 * 8 + grp * GRP: 1 * 8 + grp * GRP + GRP]
                g10 = gb[:, 2 * 8 + grp * GRP: 2 * 8 + grp * GRP + GRP]
                g11 = gb[:, 3 * 8 + grp * GRP: 3 * 8 + grp * GRP + GRP]
                br_ = gb[:, 4 * 8 + grp * GRP: 4 * 8 + grp * GRP + GRP]
                bi_ = gb[:, 5 * 8 + grp * GRP: 5 * 8 + grp * GRP + GRP]
                cbt = smallp.tile([P, 6 * GRP], f32, tag="cb")
                G00, G01, BR = qs(0, cbt), qs(1, cbt), qs(2, cbt)
                G10, G11, BI = qs(3, cbt), qs(4, cbt), qs(5, cbt)
                V.tensor_mul(q, g00, w00)
                V.tensor_mul(u, g01, w01)
                V.tensor_add(G00, q, u)
                V.tensor_mul(q, g00, w01)
                V.tensor_mul(u, g01, w11)
                V.tensor_add(G01, q, u)
                V.tensor_mul(q, g10, w00)
                V.tensor_mul(u, g11, w01)
                V.tensor_add(G10, q, u)
                V.tensor_mul(q, g10, w01)
                V.tensor_mul(u, g11, w11)
                V.tensor_add(G11, q, u)
                q2 = ts(15)
                V.tensor_mul(q, G00, MR)
                V.tensor_mul(u, G01, MI)
                V.tensor_add(q2, q, u)
                V.tensor_sub(BR, br_, q2)
                V.tensor_mul(q, G10, MR)
                V.tensor_mul(u, G11, MI)
                V.tensor_add(q2, q, u)
                V.tensor_sub(BI, bi_, q2)

                # Bounce the coefficients through DRAM so the whiten ops
                # consume a DMA-produced tile (dependency-tracked path that
                # validated on hardware).
                nc.sync.dma_start(out=scratch_d[grp:grp + 1, :], in_=cbt[0:1, :])
                cbB = smallp.tile([P, 6 * GRP], f32, tag="cbB")
                nc.sync.dma_start(
                    out=cbB[:],
                    in_=scratch_d[grp:grp + 1, :].to_broadcast((P, 6 * GRP)))

                # ---- whiten + affine: ACT does t=G*x+B, DVE adds.
                #      All operands flat packed bf16 -> 2x perf modes. ----
                for ci in range(GRP):
                    c = grp * GRP + ci
                    xt = xts[ci]
                    xr = xt[:, 0:F]
                    xi = xt[:, F:2 * F]

                    def cf(qi):
                        return cbB[:, qi * GRP + ci:qi * GRP + ci + 1]

                    yt = yp.tile([P, YW], bf16, tag="y")
                    t1 = t1p.tile([P, F], bf16, tag="t1")
                    t2 = t1p.tile([P, F], bf16, tag="t2")
                    S.activation(t1[:], xr, Act.Identity,
                                 bias=cf(2), scale=cf(0))
                    V.scalar_tensor_tensor(out=yt[:, 0:F], in0=xi,
                                           scalar=cf(1), in1=t1[:],
                                           op0=Alu.mult, op1=Alu.add)
                    S.activation(t2[:], xr, Act.Identity,
                                 bias=cf(5), scale=cf(3))
                    V.scalar_tensor_tensor(out=yt[:, F:2 * F], in0=xi,
                                           scalar=cf(4), in1=t2[:],
                                           op0=Alu.mult, op1=Alu.add)
                    nc.sync.dma_start(out=y_d[c], in_=yt[:])

    nc.finalize()
    return nc


def _get_nc():
    if "nc" not in _CACHE:
        _CACHE["nc"] = _build_nc()
    return _CACHE["nc"]


def _prep_consts():
    return np.eye(P, dtype=np.float32)


def _prep_core(x_real, x_imag, gamma, beta, k):
    c0 = k * CLOC
    xy = np.empty((CLOC, P, XYW), BF16)
    xy[:, :, 0:F] = np.ascontiguousarray(
        x_real[:, c0:c0 + CLOC].transpose(1, 0, 2, 3)
    ).reshape(CLOC, P, F).astype(BF16)
    xy[:, :, F:2 * F] = np.ascontiguousarray(
        x_imag[:, c0:c0 + CLOC].transpose(1, 0, 2, 3)
    ).reshape(CLOC, P, F).astype(BF16)
    xy[:, :, 2 * F:] = 1.0
    g = gamma[c0:c0 + CLOC]
    b = beta[c0:c0 + CLOC]
    gb = np.concatenate([g[:, 0, 0], g[:, 0, 1], g[:, 1, 0], g[:, 1, 1],
                         b[:, 0], b[:, 1]]).astype(np.float32).reshape(1, 48)
    gb = np.broadcast_to(gb, (P, 48)).copy()
    return {"xy": xy, "consts": _prep_consts(), "gb": gb}


def kernel(x_real, x_imag, gamma, beta):
    from concourse.bass_utils import run_bass_kernel_spmd

    x_real = np.asarray(x_real, dtype=np.float32)
    x_imag = np.asarray(x_imag, dtype=np.float32)
    gamma = np.asarray(gamma, dtype=np.float32)
    beta = np.asarray(beta, dtype=np.float32)

    in_maps = [_prep_core(x_real, x_imag, gamma, beta, k)
               for k in range(NCORES)]

    nc = _get_nc()
    res = None
    if _TRACE:
        try:
            res = run_bass_kernel_spmd(nc, in_maps, list(range(NCORES)),
                                       trace=True)
        except Exception as e:  # trace infra unavailable -> plain run
            LAST["trace_error"] = repr(e)
            res = None
    if res is None:
        res = run_bass_kernel_spmd(nc, in_maps, list(range(NCORES)))
    LAST["exec_time_ns"] = res.exec_time_ns
    LAST["mean_exec_time_ns"] = res.mean_exec_time_ns
    LAST["profile_json"] = res.profile_json
    if res.instructions_and_trace is not None:
        LAST["trace_path"] = res.instructions_and_trace[1]

    out = np.empty((B, C, H, W, 2), np.float32)
    for k in range(NCORES):
        c0 = k * CLOC
        y = res.results[k]["y"]  # bf16 [CLOC, P, YW], planar [re(F)|im(F)]
        y = y.reshape(CLOC, P, 2, F).astype(np.float32)
        y = y.transpose(0, 2, 1, 3).reshape(CLOC, 2, B, H, W)
        out[:, c0:c0 + CLOC] = y.transpose(2, 0, 3, 4, 1)
    return out


# revision 26
# speedup vs baseline: 1.2551x; 1.2551x over previous
"""ComplexBatchNorm2d (Trabelsi-style complex whitening BN) on 8 trn2 NeuronCores.

Sharding: over channels C (8 channels per core). Each channel's batch statistics
are computed entirely on one core, so no collectives are needed.

All HBM traffic is bf16 (the 2e-2 rel-err gate leaves ~50x headroom over bf16
quantization noise): the host downcasts inputs, the device writes bf16 planar
[re|im] outputs, and the host de-interleaves + upcasts. This halves the DMA
bytes vs fp32, which was the original bottleneck (79% DMA busy).

Channel data is PLANAR in SBUF ([X(4096) | Y(4096) | ones(2)] per channel) so
every whiten op is flat and contiguous -> DVE/ACT run in their 2x packed
16-bit modes (the interleaved-chunk layout forced 1x + per-row AP overhead).

Per-core device kernel (Bass/Tile), channel-pipelined in 2 groups of 4:
  stats:  2x2 Gram via TensorE bf16 matmuls in two flat passes per channel
          (BIR requires single-free-dim matmul operands): pass 1 loads each
          X_J 128-col chunk stationary and streams X_J (-> gXX), Y_J (-> gXY)
          and the ones column (-> X sums); pass 2 loads Y_J and streams Y_J
          (-> gYY) and ones (-> Y sums). Diagonals extracted with an
          eye128-masked TT-mult + reduce; a ones-matmul folds partitions.
  2x2:    closed-form (V + eps I)^{-1/2} batched over the 4 channels of a
          group ([P,4]-wide ops), folded with gamma/beta into
          y_re = G00*xr + G01*xi + BR (same for im); the 24 coefficients are
          broadcast to all partitions via one DRAM bounce per group.
  whiten: ScalarE (ACT) computes t = G*x + B for both components, VectorE adds
          the cross terms writing packed bf16 planar [re | im] halves; one
          contiguous 2 MB DMA out per channel.
Groups overlap: group B's loads/grams (DMA/PE) run while group A whitens
(ACT/DVE), keeping the DMA engines saturated.

Host side: slices/permutes inputs per core to planar bf16, gathers per-core
bf16 outputs and permutes back to (B, C, H, W, 2) f32.
"""

import numpy as np
import ml_dtypes

BF16 = ml_dtypes.bfloat16

# Problem geometry (hardcoded per contract).
B, C, H, W = 32, 64, 128, 128
NCORES = 8
CLOC = C // NCORES          # channels per core = 8
P = 128                     # SBUF partitions
N = B * H * W               # samples per channel = 524288
F = N // P                  # free columns per channel = 4096
CHUNK = 128                 # data columns per gram chunk (full PE width)
NCHUNK = F // CHUNK         # 32 chunks per channel
XYW = 2 * F + 2             # planar [X(F) | Y(F) | ones(2)] per channel
YW = 2 * F                  # 8192 output cols per channel: [re(F) | im(F)]
GRP = 4                     # channels per assembly group
NG = CLOC // GRP            # 2 groups
EPS = 1e-5

_CACHE = {}
_TRACE = False   # test.py sets this to capture NTFF profile / HW exec time
LAST = {}        # kernel() stores exec_time_ns etc. here

# tuning knobs (module-level so the bench harness can sweep them)
XY_BUFS = 8      # all channels prefetched: group B loads/grams overlap
                 # group A's whiten (each tile 16 KiB/partition)


def _build_nc():
    import concourse.bacc as bacc
    import concourse.mybir as mybir
    from concourse.tile import TileContext

    f32 = mybir.dt.float32
    bf16 = mybir.dt.bfloat16
    Alu = mybir.AluOpType
    Act = mybir.ActivationFunctionType
    Axis = mybir.AxisListType

    # Bacc (not raw Bass): Tile emits multi-wait sync_info that only the bacc
    # pipeline (nop/event-semaphore lowering) can legalize for walrus codegen.
    nc = bacc.Bacc("TRN2", target_bir_lowering=False)
    xy_d = nc.declare_dram_parameter("xy", [CLOC, P, XYW], bf16, isOutput=False)
    consts_d = nc.declare_dram_parameter("consts", [P, P], f32, isOutput=False)
    gb_d = nc.declare_dram_parameter("gb", [P, 48], f32, isOutput=False)
    y_d = nc.declare_dram_parameter("y", [CLOC, P, YW], bf16, isOutput=True)
    scratch_d = nc.dram_tensor("scratch", [NG, 6 * GRP], f32)

    V = nc.vector
    S = nc.scalar

    with TileContext(nc) as tc:
        with (
            tc.tile_pool(name="singles", bufs=1) as singles,
            tc.tile_pool(name="xyp", bufs=XY_BUFS) as xyp,
            tc.tile_pool(name="yp", bufs=2) as yp,
            tc.tile_pool(name="up", bufs=2) as upool,
            tc.tile_pool(name="statp", bufs=2) as statp,
            tc.tile_pool(name="smallp", bufs=2) as smallp,
            tc.tile_pool(name="gramp", bufs=2, space="PSUM") as gramp,
            tc.tile_pool(name="spsum", bufs=1, space="PSUM") as spsump,
        ):
            consts = singles.tile([P, P], f32)
            nc.sync.dma_start(out=consts[:], in_=consts_d[:])
            gb = singles.tile([P, 48], f32)
            nc.sync.dma_start(out=gb[:], in_=gb_d[:])

            # DVE-staged eye128 so the masked-diag TT ops depend on at most
            # one cross-engine producer.
            ident = singles.tile([P, P], f32)
            V.tensor_copy(ident[:], consts[:])
            # f32 ones for the partition-fold matmul of the diag partials.
            ones_f32 = singles.tile([P, P], f32)
            V.memset(ones_f32[:], 1.0)

            for grp in range(NG):
                stats = statp.tile([P, 5 * GRP], f32, tag="stats")
                xts = []
                for ci in range(GRP):
                    c = grp * GRP + ci
                    # ---- load this channel's planar data (used twice) ----
                    xt = xyp.tile([P, XYW], bf16, tag="xy")
                    nc.sync.dma_start(out=xt[:], in_=xy_d[c])
                    xts.append(xt)
                    ones_col = xt[:, 2 * F:2 * F + 1]

                    # ---- gram passes: flat 128-col chunks ----
                    # gA[:, 0:128] = X^T X (diag -> sum xr^2)
                    # gB[:, 0:128] = X^T Y (diag -> sum xr*xi)
                    # gA[:, 128:256] = Y^T Y (diag -> sum xi^2)
                    # gS[:, 0] = per-col X sums; gS[:, 1] = per-col Y sums
                    gA = gramp.tile([P, 2 * P], f32, tag="gA")
                    gB = gramp.tile([P, P], f32, tag="gB")
                    gS = gramp.tile([P, 2], f32, tag="gS")
                    for j in range(NCHUNK):
                        xj = xt[:, j * CHUNK:(j + 1) * CHUNK]
                        yj = xt[:, F + j * CHUNK:F + (j + 1) * CHUNK]
                        st = (j == 0)
                        sp = (j == NCHUNK - 1)
                        nc.tensor.matmul(gA[:, 0:P], lhsT=xj, rhs=xj,
                                         start=st, stop=sp)
                        nc.tensor.matmul(gB[:, :], lhsT=xj, rhs=yj,
                                         start=st, stop=sp)
                        nc.tensor.matmul(gS[:, 0:1], lhsT=xj, rhs=ones_col,
                                         start=st, stop=sp)
                    for j in range(NCHUNK):
                        yj = xt[:, F + j * CHUNK:F + (j + 1) * CHUNK]
                        st = (j == 0)
                        sp = (j == NCHUNK - 1)
                        nc.tensor.matmul(gA[:, P:2 * P], lhsT=yj, rhs=yj,
                                         start=st, stop=sp)
                        nc.tensor.matmul(gS[:, 1:2], lhsT=yj, rhs=ones_col,
                                         start=st, stop=sp)

                    # ---- diag extraction into the group stats tile ----
                    junk = smallp.tile([P, P], f32, tag="junk")
                    V.tensor_mul(junk[:], gA[:, 0:P], ident[:])
                    V.tensor_reduce(out=stats[:, 0 * GRP + ci:0 * GRP + ci + 1],
                                    in_=junk[:], axis=Axis.X, op=Alu.add)
                    V.tensor_mul(junk[:], gB[:, :], ident[:])
                    V.tensor_reduce(out=stats[:, 1 * GRP + ci:1 * GRP + ci + 1],
                                    in_=junk[:], axis=Axis.X, op=Alu.add)
                    V.tensor_mul(junk[:], gA[:, P:2 * P], ident[:])
                    V.tensor_reduce(out=stats[:, 2 * GRP + ci:2 * GRP + ci + 1],
                                    in_=junk[:], axis=Axis.X, op=Alu.add)
                    S.copy(stats[:, 3 * GRP + ci:3 * GRP + ci + 1], gS[:, 0:1])
                    S.copy(stats[:, 4 * GRP + ci:4 * GRP + ci + 1], gS[:, 1:2])

                # partition fold: all 128 output rows hold all 5*GRP sums
                s_ps = spsump.tile([P, 5 * GRP], f32, tag="sps")
                nc.tensor.matmul(s_ps[:, :], lhsT=ones_f32[:], rhs=stats[:],
                                 start=True, stop=True)
                s_sb = smallp.tile([P, 5 * GRP], f32, tag="ssb")
                V.tensor_copy(s_sb[:], s_ps[:, :])

                # ---- 2x2 assembly, batched over the group's GRP channels,
                #      replicated across partitions ----
                def qs(q, t=None):
                    t = s_sb if t is None else t
                    return t[:, q * GRP:(q + 1) * GRP]

                SXX, SXY, SYY = qs(0), qs(1), qs(2)
                SR, SI = qs(3), qs(4)
                tmp = smallp.tile([P, 16 * GRP], f32, tag="tmp")

                def ts(i, tmp=tmp):
                    return tmp[:, i * GRP:(i + 1) * GRP]

                rN = 1.0 / N
                rN1 = 1.0 / (N - 1)
                MR, MI, u = ts(0), ts(1), ts(2)
                a, bb, cc = ts(3), ts(4), ts(5)
                V.tensor_scalar_mul(MR, SR, rN)
                V.tensor_scalar_mul(MI, SI, rN)
                # a=(Sxx-Sx*mr)/(N-1)+eps; b=(Sxy-Sx*mi)/(N-1);
                # c=(Syy-Sy*mi)/(N-1)+eps
                V.tensor_mul(u, SR, MR)
                V.tensor_sub(a, SXX, u)
                V.tensor_scalar(out=a, in0=a, scalar1=rN1, scalar2=EPS,
                                op0=Alu.mult, op1=Alu.add)
                V.tensor_mul(u, SR, MI)
                V.tensor_sub(bb, SXY, u)
                V.tensor_scalar_mul(bb, bb, rN1)
                V.tensor_mul(u, SI, MI)
                V.tensor_sub(cc, SYY, u)
                V.tensor_scalar(out=cc, in0=cc, scalar1=rN1, scalar2=EPS,
                                op0=Alu.mult, op1=Alu.add)
                # (M)^{-1/2} for M=[[a,b],[b,c]]: s=sqrt(ac-b^2);
                # t=sqrt(a+c+2s); W=[[c+s,-b],[-b,a+s]]/(s*t)
                det, s_, tr, st_, inv = ts(6), ts(7), ts(8), ts(9), ts(10)
                V.tensor_mul(det, a, cc)
                V.tensor_mul(u, bb, bb)
                V.tensor_sub(det, det, u)
                nc.scalar.sqrt(s_, det)
                V.tensor_add(u, a, cc)
                V.tensor_scalar_mul(tr, s_, 2.0)
                V.tensor_add(tr, tr, u)
                nc.scalar.sqrt(tr, tr)
                V.tensor_mul(st_, s_, tr)
                V.reciprocal(inv, st_)
                w00, w01, w11, q = ts(11), ts(12), ts(13), ts(14)
                V.tensor_add(w00, cc, s_)
                V.tensor_mul(w00, w00, inv)
                V.scalar_tensor_tensor(out=w01, in0=bb, scalar=-1.0, in1=inv,
                                       op0=Alu.mult, op1=Alu.mult)
                V.tensor_add(w11, a, s_)
                V.tensor_mul(w11, w11, inv)
                # G = gamma @ W ; B' = beta - G @ mean  (gb is quantity-major
                # over 8 channels; this group's 4 are contiguous)
                g00 = gb[:, 0 * 8 + grp * GRP: 0 * 8 + grp * GRP + GRP]
                g01 = gb[:, 1 * 8 + grp * GRP: 1 * 8 + grp * GRP + GRP]
                g10 = gb[:, 2 * 8 + grp * GRP: 2 * 8 + grp * GRP + GRP]
                g11 = gb[:, 3 * 8 + grp * GRP: 3 * 8 + grp * GRP + GRP]
                br_ = gb[:, 4 * 8 + grp * GRP: 4 * 8 + grp * GRP + GRP]
                bi_ = gb[:, 5 * 8 + grp * GRP: 5 * 8 + grp * GRP + GRP]
                cbt = smallp.tile([P, 6 * GRP], f32, tag="cb")
                G00, G01, BR = qs(0, cbt), qs(1, cbt), qs(2, cbt)
                G10, G11, BI = qs(3, cbt), qs(4, cbt), qs(5, cbt)
                V.tensor_mul(q, g00, w00)
                V.tensor_mul(u, g01, w01)
                V.tensor_add(G00, q, u)
                V.tensor_mul(q, g00, w01)
                V.tensor_mul(u, g01, w11)
                V.tensor_add(G01, q, u)
                V.tensor_mul(q, g10, w00)
                V.tensor_mul(u, g11, w01)
                V.tensor_add(G10, q, u)
                V.tensor_mul(q, g10, w01)
                V.tensor_mul(u, g11, w11)
                V.tensor_add(G11, q, u)
                q2 = ts(15)
                V.tensor_mul(q, G00, MR)
                V.tensor_mul(u, G01, MI)
                V.tensor_add(q2, q, u)
                V.tensor_sub(BR, br_, q2)
                V.tensor_mul(q, G10, MR)
                V.tensor_mul(u, G11, MI)
                V.tensor_add(q2, q, u)
                V.tensor_sub(BI, bi_, q2)

                # Bounce the coefficients through DRAM so the whiten ops
                # consume a DMA-produced tile (dependency-tracked path that
                # validated on hardware). Per-partition scalar operands are
                # exempt from the 2-byte packed-mode operand checks.
                nc.sync.dma_start(out=scratch_d[grp:grp + 1, :], in_=cbt[0:1, :])
                cbB = smallp.tile([P, 6 * GRP], f32, tag="cbB")
                nc.sync.dma_start(
                    out=cbB[:],
                    in_=scratch_d[grp:grp + 1, :].to_broadcast((P, 6 * GRP)))

                # ---- whiten + affine: ACT writes t=G*x+B straight into the
                #      output tile, DVE computes u=G'*xi (tensor_scalar, 4x
                #      eligible) and adds it in place (tensor_tensor, 2x
                #      eligible). All tensor operands flat packed bf16. ----
                for ci in range(GRP):
                    c = grp * GRP + ci
                    xt = xts[ci]
                    xr = xt[:, 0:F]
                    xi = xt[:, F:2 * F]

                    def cf(qi):
                        return cbB[:, qi * GRP + ci:qi * GRP + ci + 1]

                    yre = yp.tile([P, F], bf16, tag="y")
                    u1 = upool.tile([P, F], bf16, tag="u")
                    S.activation(yre[:], xr, Act.Identity,
                                 bias=cf(2), scale=cf(0))
                    V.tensor_scalar_mul(u1[:], xi, cf(1))
                    V.tensor_add(yre[:], yre[:], u1[:])
                    nc.sync.dma_start(out=y_d[c][:, 0:F], in_=yre[:])
                    yim = yp.tile([P, F], bf16, tag="y")
                    u2 = upool.tile([P, F], bf16, tag="u")
                    S.activation(yim[:], xr, Act.Identity,
                                 bias=cf(5), scale=cf(3))
                    V.tensor_scalar_mul(u2[:], xi, cf(4))
                    V.tensor_add(yim[:], yim[:], u2[:])
                    nc.sync.dma_start(out=y_d[c][:, F:2 * F], in_=yim[:])

    nc.finalize()
    return nc


def _get_nc():
    if "nc" not in _CACHE:
        _CACHE["nc"] = _build_nc()
    return _CACHE["nc"]


def _prep_consts():
    return np.eye(P, dtype=np.float32)


def _prep_core(x_real, x_imag, gamma, beta, k):
    c0 = k * CLOC
    xy = np.empty((CLOC, P, XYW), BF16)
    xy[:, :, 0:F] = np.ascontiguousarray(
        x_real[:, c0:c0 + CLOC].transpose(1, 0, 2, 3)
    ).reshape(CLOC, P, F).astype(BF16)
    xy[:, :, F:2 * F] = np.ascontiguousarray(
        x_imag[:, c0:c0 + CLOC].transpose(1, 0, 2, 3)
    ).reshape(CLOC, P, F).astype(BF16)
    xy[:, :, 2 * F:] = 1.0
    g = gamma[c0:c0 + CLOC]
    b = beta[c0:c0 + CLOC]
    gb = np.concatenate([g[:, 0, 0], g[:, 0, 1], g[:, 1, 0], g[:, 1, 1],
                         b[:, 0], b[:, 1]]).astype(np.float32).reshape(1, 48)
    gb = np.broadcast_to(gb, (P, 48)).copy()
    return {"xy": xy, "consts": _prep_consts(), "gb": gb}


def kernel(x_real, x_imag, gamma, beta):
    from concourse.bass_utils import run_bass_kernel_spmd

    x_real = np.asarray(x_real, dtype=np.float32)
    x_imag = np.asarray(x_imag, dtype=np.float32)
    gamma = np.asarray(gamma, dtype=np.float32)
    beta = np.asarray(beta, dtype=np.float32)

    in_maps = [_prep_core(x_real, x_imag, gamma, beta, k)
               for k in range(NCORES)]

    nc = _get_nc()
    res = None
    if _TRACE:
        try:
            res = run_bass_kernel_spmd(nc, in_maps, list(range(NCORES)),
                                       trace=True)
        except Exception as e:  # trace infra unavailable -> plain run
            LAST["trace_error"] = repr(e)
            res = None
    if res is None:
        res = run_bass_kernel_spmd(nc, in_maps, list(range(NCORES)))
    LAST["exec_time_ns"] = res.exec_time_ns
    LAST["mean_exec_time_ns"] = res.mean_exec_time_ns
    LAST["profile_json"] = res.profile_json
    if res.instructions_and_trace is not None:
        LAST["trace_path"] = res.instructions_and_trace[1]

    out = np.empty((B, C, H, W, 2), np.float32)
    for k in range(NCORES):
        c0 = k * CLOC
        y = res.results[k]["y"]  # bf16 [CLOC, P, YW], planar [re(F)|im(F)]
        y = y.reshape(CLOC, P, 2, F).astype(np.float32)
        y = y.transpose(0, 2, 1, 3).reshape(CLOC, 2, B, H, W)
        out[:, c0:c0 + CLOC] = y.transpose(2, 0, 3, 4, 1)
    return out


# revision 29
# speedup vs baseline: 1.4226x; 1.1335x over previous
"""ComplexBatchNorm2d (Trabelsi-style complex whitening BN) on 8 trn2 NeuronCores.

Sharding: over channels C (8 channels per core). Each channel's batch statistics
are computed entirely on one core, so no collectives are needed.

All HBM traffic is bf16 (the 2e-2 rel-err gate leaves ~50x headroom over bf16
quantization noise): the host downcasts inputs, the device writes bf16 planar
[re|im] outputs, and the host de-interleaves + upcasts. This halves the DMA
bytes vs fp32, which was the original bottleneck (79% DMA busy).

Channel data is PLANAR in SBUF ([X(4096) | Y(4096) | ones(2)] per channel) so
every whiten op is flat and contiguous -> DVE/ACT run in their 2x packed
16-bit modes (the interleaved-chunk layout forced 1x + per-row AP overhead).

Per-core device kernel (Bass/Tile), channel-pipelined in 2 groups of 4:
  stats:  2x2 Gram via TensorE bf16 matmuls in two flat passes per channel
          (BIR requires single-free-dim matmul operands): pass 1 loads each
          X_J 128-col chunk stationary and streams X_J (-> gXX), Y_J (-> gXY)
          and the ones column (-> X sums); pass 2 loads Y_J and streams Y_J
          (-> gYY) and ones (-> Y sums). Diagonals extracted with an
          eye128-masked TT-mult + reduce; a ones-matmul folds partitions.
  2x2:    closed-form (V + eps I)^{-1/2} batched over the 4 channels of a
          group ([P,4]-wide ops), folded with gamma/beta into
          y_re = G00*xr + G01*xi + BR (same for im); the 24 coefficients are
          broadcast to all partitions via one DRAM bounce per group.
  whiten: ScalarE (ACT) computes t = G*x + B for both components, VectorE adds
          the cross terms writing packed bf16 planar [re | im] halves; one
          contiguous 2 MB DMA out per channel.
Groups overlap: group B's loads/grams (DMA/PE) run while group A whitens
(ACT/DVE), keeping the DMA engines saturated.

Host side: slices/permutes inputs per core to planar bf16, gathers per-core
bf16 outputs and permutes back to (B, C, H, W, 2) f32.
"""

import numpy as np
import ml_dtypes

BF16 = ml_dtypes.bfloat16

# Problem geometry (hardcoded per contract).
B, C, H, W = 32, 64, 128, 128
NCORES = 8
CLOC = C // NCORES          # channels per core = 8
P = 128                     # SBUF partitions
N = B * H * W               # samples per channel = 524288
F = N // P                  # free columns per channel = 4096
CHUNK = 128                 # data columns per gram chunk (full PE width)
NCHUNK = F // CHUNK         # 32 chunks per channel
XYW = 2 * F + 2             # planar [X(F) | Y(F) | ones(2)] per channel
YW = 2 * F                  # 8192 output cols per channel: [re(F) | im(F)]
GRP = 4                     # channels per assembly group
NG = CLOC // GRP            # 2 groups
EPS = 1e-5

_CACHE = {}
_TRACE = False   # test.py sets this to capture NTFF profile / HW exec time
LAST = {}        # kernel() stores exec_time_ns etc. here

# tuning knobs (module-level so the bench harness can sweep them)
XY_BUFS = 7      # near-all channels prefetched: group B loads/grams overlap
                 # group A's whiten (each tile 16 KiB/partition)


def _build_nc():
    import concourse.bacc as bacc
    import concourse.mybir as mybir
    from concourse.tile import TileContext

    f32 = mybir.dt.float32
    bf16 = mybir.dt.bfloat16
    Alu = mybir.AluOpType
    Act = mybir.ActivationFunctionType
    Axis = mybir.AxisListType

    # Bacc (not raw Bass): Tile emits multi-wait sync_info that only the bacc
    # pipeline (nop/event-semaphore lowering) can legalize for walrus codegen.
    nc = bacc.Bacc("TRN2", target_bir_lowering=False)
    xy_d = nc.declare_dram_parameter("xy", [CLOC, P, XYW], bf16, isOutput=False)
    consts_d = nc.declare_dram_parameter("consts", [P, P], f32, isOutput=False)
    gb_d = nc.declare_dram_parameter("gb", [P, 48], f32, isOutput=False)
    y_d = nc.declare_dram_parameter("y", [CLOC, P, YW], bf16, isOutput=True)
    scratch_d = nc.dram_tensor("scratch", [NG, 6 * GRP], f32)

    V = nc.vector
    S = nc.scalar

    with TileContext(nc) as tc:
        with (
            tc.tile_pool(name="singles", bufs=1) as singles,
            tc.tile_pool(name="xyp", bufs=XY_BUFS) as xyp,
            tc.tile_pool(name="yp", bufs=4) as yp,
            tc.tile_pool(name="up", bufs=4) as upool,
            tc.tile_pool(name="statp", bufs=2) as statp,
            tc.tile_pool(name="smallp", bufs=2) as smallp,
            tc.tile_pool(name="gramp", bufs=2, space="PSUM") as gramp,
            tc.tile_pool(name="spsum", bufs=1, space="PSUM") as spsump,
        ):
            consts = singles.tile([P, P], f32)
            nc.sync.dma_start(out=consts[:], in_=consts_d[:])
            gb = singles.tile([P, 48], f32)
            nc.sync.dma_start(out=gb[:], in_=gb_d[:])

            # DVE-staged eye128 so the masked-diag TT ops depend on at most
            # one cross-engine producer.
            ident = singles.tile([P, P], f32)
            V.tensor_copy(ident[:], consts[:])
            # f32 ones for the partition-fold matmul of the diag partials.
            ones_f32 = singles.tile([P, P], f32)
            V.memset(ones_f32[:], 1.0)

            for grp in range(NG):
                stats = statp.tile([P, 5 * GRP], f32, tag="stats")
                xts = []
                for ci in range(GRP):
                    c = grp * GRP + ci
                    # ---- load this channel's planar data (used twice) ----
                    xt = xyp.tile([P, XYW], bf16, tag="xy")
                    nc.sync.dma_start(out=xt[:], in_=xy_d[c])
                    xts.append(xt)
                    ones_col = xt[:, 2 * F:2 * F + 1]

                    # ---- gram passes: flat 128-col chunks ----
                    # gA[:, 0:128] = X^T X (diag -> sum xr^2)
                    # gB[:, 0:128] = X^T Y (diag -> sum xr*xi)
                    # gA[:, 128:256] = Y^T Y (diag -> sum xi^2)
                    # gS[:, 0] = per-col X sums; gS[:, 1] = per-col Y sums
                    gA = gramp.tile([P, 2 * P], f32, tag="gA")
                    gB = gramp.tile([P, P], f32, tag="gB")
                    gS = gramp.tile([P, 2], f32, tag="gS")
                    for j in range(NCHUNK):
                        xj = xt[:, j * CHUNK:(j + 1) * CHUNK]
                        yj = xt[:, F + j * CHUNK:F + (j + 1) * CHUNK]
                        st = (j == 0)
                        sp = (j == NCHUNK - 1)
                        nc.tensor.matmul(gA[:, 0:P], lhsT=xj, rhs=xj,
                                         start=st, stop=sp)
                        nc.tensor.matmul(gB[:, :], lhsT=xj, rhs=yj,
                                         start=st, stop=sp)
                        nc.tensor.matmul(gS[:, 0:1], lhsT=xj, rhs=ones_col,
                                         start=st, stop=sp)
                    for j in range(NCHUNK):
                        yj = xt[:, F + j * CHUNK:F + (j + 1) * CHUNK]
                        st = (j == 0)
                        sp = (j == NCHUNK - 1)
                        nc.tensor.matmul(gA[:, P:2 * P], lhsT=yj, rhs=yj,
                                         start=st, stop=sp)
                        nc.tensor.matmul(gS[:, 1:2], lhsT=yj, rhs=ones_col,
                                         start=st, stop=sp)

                    # ---- diag extraction into the group stats tile ----
                    junk = smallp.tile([P, P], f32, tag="junk")
                    V.tensor_mul(junk[:], gA[:, 0:P], ident[:])
                    V.tensor_reduce(out=stats[:, 0 * GRP + ci:0 * GRP + ci + 1],
                                    in_=junk[:], axis=Axis.X, op=Alu.add)
                    V.tensor_mul(junk[:], gB[:, :], ident[:])
                    V.tensor_reduce(out=stats[:, 1 * GRP + ci:1 * GRP + ci + 1],
                                    in_=junk[:], axis=Axis.X, op=Alu.add)
                    V.tensor_mul(junk[:], gA[:, P:2 * P], ident[:])
                    V.tensor_reduce(out=stats[:, 2 * GRP + ci:2 * GRP + ci + 1],
                                    in_=junk[:], axis=Axis.X, op=Alu.add)
                    S.copy(stats[:, 3 * GRP + ci:3 * GRP + ci + 1], gS[:, 0:1])
                    S.copy(stats[:, 4 * GRP + ci:4 * GRP + ci + 1], gS[:, 1:2])

                # partition fold: all 128 output rows hold all 5*GRP sums
                s_ps = spsump.tile([P, 5 * GRP], f32, tag="sps")
                nc.tensor.matmul(s_ps[:, :], lhsT=ones_f32[:], rhs=stats[:],
                                 start=True, stop=True)
                s_sb = smallp.tile([P, 5 * GRP], f32, tag="ssb")
                V.tensor_copy(s_sb[:], s_ps[:, :])

                # ---- 2x2 assembly, batched over the group's GRP channels,
                #      replicated across partitions ----
                def qs(q, t=None):
                    t = s_sb if t is None else t
                    return t[:, q * GRP:(q + 1) * GRP]

                SXX, SXY, SYY = qs(0), qs(1), qs(2)
                SR, SI = qs(3), qs(4)
                tmp = smallp.tile([P, 16 * GRP], f32, tag="tmp")

                def ts(i, tmp=tmp):
                    return tmp[:, i * GRP:(i + 1) * GRP]

                rN = 1.0 / N
                rN1 = 1.0 / (N - 1)
                MR, MI, u = ts(0), ts(1), ts(2)
                a, bb, cc = ts(3), ts(4), ts(5)
                V.tensor_scalar_mul(MR, SR, rN)
                V.tensor_scalar_mul(MI, SI, rN)
                # a=(Sxx-Sx*mr)/(N-1)+eps; b=(Sxy-Sx*mi)/(N-1);
                # c=(Syy-Sy*mi)/(N-1)+eps
                V.tensor_mul(u, SR, MR)
                V.tensor_sub(a, SXX, u)
                V.tensor_scalar(out=a, in0=a, scalar1=rN1, scalar2=EPS,
                                op0=Alu.mult, op1=Alu.add)
                V.tensor_mul(u, SR, MI)
                V.tensor_sub(bb, SXY, u)
                V.tensor_scalar_mul(bb, bb, rN1)
                V.tensor_mul(u, SI, MI)
                V.tensor_sub(cc, SYY, u)
                V.tensor_scalar(out=cc, in0=cc, scalar1=rN1, scalar2=EPS,
                                op0=Alu.mult, op1=Alu.add)
                # (M)^{-1/2} for M=[[a,b],[b,c]]: s=sqrt(ac-b^2);
                # t=sqrt(a+c+2s); W=[[c+s,-b],[-b,a+s]]/(s*t)
                det, s_, tr, st_, inv = ts(6), ts(7), ts(8), ts(9), ts(10)
                V.tensor_mul(det, a, cc)
                V.tensor_mul(u, bb, bb)
                V.tensor_sub(det, det, u)
                nc.scalar.sqrt(s_, det)
                V.tensor_add(u, a, cc)
                V.tensor_scalar_mul(tr, s_, 2.0)
                V.tensor_add(tr, tr, u)
                nc.scalar.sqrt(tr, tr)
                V.tensor_mul(st_, s_, tr)
                V.reciprocal(inv, st_)
                w00, w01, w11, q = ts(11), ts(12), ts(13), ts(14)
                V.tensor_add(w00, cc, s_)
                V.tensor_mul(w00, w00, inv)
                V.scalar_tensor_tensor(out=w01, in0=bb, scalar=-1.0, in1=inv,
                                       op0=Alu.mult, op1=Alu.mult)
                V.tensor_add(w11, a, s_)
                V.tensor_mul(w11, w11, inv)
                # G = gamma @ W ; B' = beta - G @ mean  (gb is quantity-major
                # over 8 channels; this group's 4 are contiguous)
                g00 = gb[:, 0 * 8 + grp * GRP: 0 * 8 + grp * GRP + GRP]
                g01 = gb[:, 1 * 8 + grp * GRP: 1 * 8 + grp * GRP + GRP]
                g10 = gb[:, 2 * 8 + grp * GRP: 2 * 8 + grp * GRP + GRP]
                g11 = gb[:, 3 * 8 + grp * GRP: 3 * 8 + grp * GRP + GRP]
                br_ = gb[:, 4 * 8 + grp * GRP: 4 * 8 + grp * GRP + GRP]
                bi_ = gb[:, 5 * 8 + grp * GRP: 5 * 8 + grp * GRP + GRP]
                cbt = smallp.tile([P, 6 * GRP], f32, tag="cb")
                G00, G01, BR = qs(0, cbt), qs(1, cbt), qs(2, cbt)
                G10, G11, BI = qs(3, cbt), qs(4, cbt), qs(5, cbt)
                V.tensor_mul(q, g00, w00)
                V.tensor_mul(u, g01, w01)
                V.tensor_add(G00, q, u)
                V.tensor_mul(q, g00, w01)
                V.tensor_mul(u, g01, w11)
                V.tensor_add(G01, q, u)
                V.tensor_mul(q, g10, w00)
                V.tensor_mul(u, g11, w01)
                V.tensor_add(G10, q, u)
                V.tensor_mul(q, g10, w01)
                V.tensor_mul(u, g11, w11)
                V.tensor_add(G11, q, u)
                q2 = ts(15)
                V.tensor_mul(q, G00, MR)
                V.tensor_mul(u, G01, MI)
                V.tensor_add(q2, q, u)
                V.tensor_sub(BR, br_, q2)
                V.tensor_mul(q, G10, MR)
                V.tensor_mul(u, G11, MI)
                V.tensor_add(q2, q, u)
                V.tensor_sub(BI, bi_, q2)

                # Bounce the coefficients through DRAM so the whiten ops
                # consume a DMA-produced tile (dependency-tracked path that
                # validated on hardware). Per-partition scalar operands are
                # exempt from the 2-byte packed-mode operand checks.
                nc.sync.dma_start(out=scratch_d[grp:grp + 1, :], in_=cbt[0:1, :])
                cbB = smallp.tile([P, 6 * GRP], f32, tag="cbB")
                nc.sync.dma_start(
                    out=cbB[:],
                    in_=scratch_d[grp:grp + 1, :].to_broadcast((P, 6 * GRP)))

                # ---- whiten + affine: ACT writes t=G*x+B straight into the
                #      output tile, DVE computes u=G'*xi (tensor_scalar, 4x
                #      eligible) and adds it in place (tensor_tensor, 2x
                #      eligible). All tensor operands flat packed bf16. ----
                for ci in range(GRP):
                    c = grp * GRP + ci
                    xt = xts[ci]
                    xr = xt[:, 0:F]
                    xi = xt[:, F:2 * F]

                    def cf(qi):
                        return cbB[:, qi * GRP + ci:qi * GRP + ci + 1]

                    yre = yp.tile([P, F], bf16, tag="y")
                    yim = yp.tile([P, F], bf16, tag="y")
                    u1 = upool.tile([P, F], bf16, tag="u")
                    u2 = upool.tile([P, F], bf16, tag="u")
                    # Both ACTs back-to-back (dense ScalarE queue), DVE's
                    # u-products run concurrently, then the two in-place
                    # adds, then the stores.
                    S.activation(yre[:], xr, Act.Identity,
                                 bias=cf(2), scale=cf(0))
                    S.activation(yim[:], xr, Act.Identity,
                                 bias=cf(5), scale=cf(3))
                    V.tensor_scalar_mul(u1[:], xi, cf(1))
                    V.tensor_scalar_mul(u2[:], xi, cf(4))
                    V.tensor_add(yre[:], yre[:], u1[:])
                    V.tensor_add(yim[:], yim[:], u2[:])
                    nc.sync.dma_start(out=y_d[c][:, 0:F], in_=yre[:])
                    nc.sync.dma_start(out=y_d[c][:, F:2 * F], in_=yim[:])

    nc.finalize()
    return nc


def _get_nc():
    if "nc" not in _CACHE:
        _CACHE["nc"] = _build_nc()
    return _CACHE["nc"]


def _prep_consts():
    return np.eye(P, dtype=np.float32)


def _prep_core(x_real, x_imag, gamma, beta, k):
    c0 = k * CLOC
    xy = np.empty((CLOC, P, XYW), BF16)
    xy[:, :, 0:F] = np.ascontiguousarray(
        x_real[:, c0:c0 + CLOC].transpose(1, 0, 2, 3)
    ).reshape(CLOC, P, F).astype(BF16)
    xy[:, :, F:2 * F] = np.ascontiguousarray(
        x_imag[:, c0:c0 + CLOC].transpose(1, 0, 2, 3)
    ).reshape(CLOC, P, F).astype(BF16)
    xy[:, :, 2 * F:] = 1.0
    g = gamma[c0:c0 + CLOC]
    b = beta[c0:c0 + CLOC]
    gb = np.concatenate([g[:, 0, 0], g[:, 0, 1], g[:, 1, 0], g[:, 1, 1],
                         b[:, 0], b[:, 1]]).astype(np.float32).reshape(1, 48)
    gb = np.broadcast_to(gb, (P, 48)).copy()
    return {"xy": xy, "consts": _prep_consts(), "gb": gb}


def kernel(x_real, x_imag, gamma, beta):
    from concourse.bass_utils import run_bass_kernel_spmd

    x_real = np.asarray(x_real, dtype=np.float32)
    x_imag = np.asarray(x_imag, dtype=np.float32)
    gamma = np.asarray(gamma, dtype=np.float32)
    beta = np.asarray(beta, dtype=np.float32)

    in_maps = [_prep_core(x_real, x_imag, gamma, beta, k)
               for k in range(NCORES)]

    nc = _get_nc()
    res = None
    if _TRACE:
        try:
            res = run_bass_kernel_spmd(nc, in_maps, list(range(NCORES)),
                                       trace=True)
        except Exception as e:  # trace infra unavailable -> plain run
            LAST["trace_error"] = repr(e)
            res = None
    if res is None:
        res = run_bass_kernel_spmd(nc, in_maps, list(range(NCORES)))
    LAST["exec_time_ns"] = res.exec_time_ns
    LAST["mean_exec_time_ns"] = res.mean_exec_time_ns
    LAST["profile_json"] = res.profile_json
    if res.instructions_and_trace is not None:
        LAST["trace_path"] = res.instructions_and_trace[1]

    out = np.empty((B, C, H, W, 2), np.float32)
    for k in range(NCORES):
        c0 = k * CLOC
        y = res.results[k]["y"]  # bf16 [CLOC, P, YW], planar [re(F)|im(F)]
        y = y.reshape(CLOC, P, 2, F).astype(np.float32)
        y = y.transpose(0, 2, 1, 3).reshape(CLOC, 2, B, H, W)
        out[:, c0:c0 + CLOC] = y.transpose(2, 0, 3, 4, 1)
    return out


# revision 30
# speedup vs baseline: 1.5026x; 1.0562x over previous
"""ComplexBatchNorm2d (Trabelsi-style complex whitening BN) on 8 trn2 NeuronCores.

Sharding: over channels C (8 channels per core). Each channel's batch statistics
are computed entirely on one core, so no collectives are needed.

All HBM traffic is bf16 (the 2e-2 rel-err gate leaves ~50x headroom over bf16
quantization noise): the host downcasts inputs, the device writes bf16 planar
[re|im] outputs, and the host de-interleaves + upcasts. This halves the DMA
bytes vs fp32, which was the original bottleneck (79% DMA busy).

Channel data is PLANAR in SBUF ([X(4096) | Y(4096) | ones(2)] per channel) so
every whiten op is flat and contiguous -> DVE/ACT run in their 2x packed
16-bit modes (the interleaved-chunk layout forced 1x + per-row AP overhead).

Per-core device kernel (Bass/Tile), channel-pipelined in 2 groups of 4:
  stats:  2x2 Gram via TensorE bf16 matmuls in two flat passes per channel
          (BIR requires single-free-dim matmul operands): pass 1 loads each
          X_J 128-col chunk stationary and streams X_J (-> gXX), Y_J (-> gXY)
          and the ones column (-> X sums); pass 2 loads Y_J and streams Y_J
          (-> gYY) and ones (-> Y sums). Diagonals extracted with an
          eye128-masked TT-mult + reduce; a ones-matmul folds partitions.
  2x2:    closed-form (V + eps I)^{-1/2} batched over the 4 channels of a
          group ([P,4]-wide ops), folded with gamma/beta into
          y_re = G00*xr + G01*xi + BR (same for im); the 24 coefficients are
          broadcast to all partitions via one DRAM bounce per group.
  whiten: ScalarE (ACT) computes t = G*x + B for both components, VectorE adds
          the cross terms writing packed bf16 planar [re | im] halves; one
          contiguous 2 MB DMA out per channel.
Groups overlap: group B's loads/grams (DMA/PE) run while group A whitens
(ACT/DVE), keeping the DMA engines saturated.

Host side: slices/permutes inputs per core to planar bf16, gathers per-core
bf16 outputs and permutes back to (B, C, H, W, 2) f32.
"""

import numpy as np
import ml_dtypes

BF16 = ml_dtypes.bfloat16

# Problem geometry (hardcoded per contract).
B, C, H, W = 32, 64, 128, 128
NCORES = 8
CLOC = C // NCORES          # channels per core = 8
P = 128                     # SBUF partitions
N = B * H * W               # samples per channel = 524288
F = N // P                  # free columns per channel = 4096
CHUNK = 128                 # data columns per gram chunk (full PE width)
NCHUNK = F // CHUNK         # 32 chunks per channel
XYW = 2 * F + 2             # planar [X(F) | Y(F) | ones(2)] per channel
YW = 2 * F                  # 8192 output cols per channel: [re(F) | im(F)]
GRP = 2                     # channels per assembly group
NG = CLOC // GRP            # 2 groups
EPS = 1e-5

_CACHE = {}
_TRACE = False   # test.py sets this to capture NTFF profile / HW exec time
LAST = {}        # kernel() stores exec_time_ns etc. here

# tuning knobs (module-level so the bench harness can sweep them)
XY_BUFS = 7      # near-all channels prefetched: group B loads/grams overlap
                 # group A's whiten (each tile 16 KiB/partition)


def _build_nc():
    import concourse.bacc as bacc
    import concourse.mybir as mybir
    from concourse.tile import TileContext

    f32 = mybir.dt.float32
    bf16 = mybir.dt.bfloat16
    Alu = mybir.AluOpType
    Act = mybir.ActivationFunctionType
    Axis = mybir.AxisListType

    # Bacc (not raw Bass): Tile emits multi-wait sync_info that only the bacc
    # pipeline (nop/event-semaphore lowering) can legalize for walrus codegen.
    nc = bacc.Bacc("TRN2", target_bir_lowering=False)
    xy_d = nc.declare_dram_parameter("xy", [CLOC, P, XYW], bf16, isOutput=False)
    consts_d = nc.declare_dram_parameter("consts", [P, P], f32, isOutput=False)
    gb_d = nc.declare_dram_parameter("gb", [P, 48], f32, isOutput=False)
    y_d = nc.declare_dram_parameter("y", [CLOC, P, YW], bf16, isOutput=True)
    scratch_d = nc.dram_tensor("scratch", [NG, 6 * GRP], f32)

    V = nc.vector
    S = nc.scalar

    with TileContext(nc) as tc:
        with (
            tc.tile_pool(name="singles", bufs=1) as singles,
            tc.tile_pool(name="xyp", bufs=XY_BUFS) as xyp,
            tc.tile_pool(name="yp", bufs=4) as yp,
            tc.tile_pool(name="up", bufs=4) as upool,
            tc.tile_pool(name="statp", bufs=2) as statp,
            tc.tile_pool(name="smallp", bufs=2) as smallp,
            tc.tile_pool(name="gramp", bufs=2, space="PSUM") as gramp,
            tc.tile_pool(name="spsum", bufs=1, space="PSUM") as spsump,
        ):
            consts = singles.tile([P, P], f32)
            nc.sync.dma_start(out=consts[:], in_=consts_d[:])
            gb = singles.tile([P, 48], f32)
            nc.sync.dma_start(out=gb[:], in_=gb_d[:])

            # DVE-staged eye128 so the masked-diag TT ops depend on at most
            # one cross-engine producer.
            ident = singles.tile([P, P], f32)
            V.tensor_copy(ident[:], consts[:])
            # f32 ones for the partition-fold matmul of the diag partials.
            ones_f32 = singles.tile([P, P], f32)
            V.memset(ones_f32[:], 1.0)

            for grp in range(NG):
                stats = statp.tile([P, 5 * GRP], f32, tag="stats")
                xts = []
                for ci in range(GRP):
                    c = grp * GRP + ci
                    # ---- load this channel's planar data (used twice) ----
                    xt = xyp.tile([P, XYW], bf16, tag="xy")
                    nc.sync.dma_start(out=xt[:], in_=xy_d[c])
                    xts.append(xt)
                    ones_col = xt[:, 2 * F:2 * F + 1]

                    # ---- gram passes: flat 128-col chunks ----
                    # gA[:, 0:128] = X^T X (diag -> sum xr^2)
                    # gB[:, 0:128] = X^T Y (diag -> sum xr*xi)
                    # gA[:, 128:256] = Y^T Y (diag -> sum xi^2)
                    # gS[:, 0] = per-col X sums; gS[:, 1] = per-col Y sums
                    gA = gramp.tile([P, 2 * P], f32, tag="gA")
                    gB = gramp.tile([P, P], f32, tag="gB")
                    gS = gramp.tile([P, 2], f32, tag="gS")
                    for j in range(NCHUNK):
                        xj = xt[:, j * CHUNK:(j + 1) * CHUNK]
                        yj = xt[:, F + j * CHUNK:F + (j + 1) * CHUNK]
                        st = (j == 0)
                        sp = (j == NCHUNK - 1)
                        nc.tensor.matmul(gA[:, 0:P], lhsT=xj, rhs=xj,
                                         start=st, stop=sp)
                        nc.tensor.matmul(gB[:, :], lhsT=xj, rhs=yj,
                                         start=st, stop=sp)
                        nc.tensor.matmul(gS[:, 0:1], lhsT=xj, rhs=ones_col,
                                         start=st, stop=sp)
                    for j in range(NCHUNK):
                        yj = xt[:, F + j * CHUNK:F + (j + 1) * CHUNK]
                        st = (j == 0)
                        sp = (j == NCHUNK - 1)
                        nc.tensor.matmul(gA[:, P:2 * P], lhsT=yj, rhs=yj,
                                         start=st, stop=sp)
                        nc.tensor.matmul(gS[:, 1:2], lhsT=yj, rhs=ones_col,
                                         start=st, stop=sp)

                    # ---- diag extraction into the group stats tile ----
                    junk = smallp.tile([P, P], f32, tag="junk")
                    V.tensor_mul(junk[:], gA[:, 0:P], ident[:])
                    V.tensor_reduce(out=stats[:, 0 * GRP + ci:0 * GRP + ci + 1],
                                    in_=junk[:], axis=Axis.X, op=Alu.add)
                    V.tensor_mul(junk[:], gB[:, :], ident[:])
                    V.tensor_reduce(out=stats[:, 1 * GRP + ci:1 * GRP + ci + 1],
                                    in_=junk[:], axis=Axis.X, op=Alu.add)
                    V.tensor_mul(junk[:], gA[:, P:2 * P], ident[:])
                    V.tensor_reduce(out=stats[:, 2 * GRP + ci:2 * GRP + ci + 1],
                                    in_=junk[:], axis=Axis.X, op=Alu.add)
                    S.copy(stats[:, 3 * GRP + ci:3 * GRP + ci + 1], gS[:, 0:1])
                    S.copy(stats[:, 4 * GRP + ci:4 * GRP + ci + 1], gS[:, 1:2])

                # partition fold: all 128 output rows hold all 5*GRP sums
                s_ps = spsump.tile([P, 5 * GRP], f32, tag="sps")
                nc.tensor.matmul(s_ps[:, :], lhsT=ones_f32[:], rhs=stats[:],
                                 start=True, stop=True)
                s_sb = smallp.tile([P, 5 * GRP], f32, tag="ssb")
                V.tensor_copy(s_sb[:], s_ps[:, :])

                # ---- 2x2 assembly, batched over the group's GRP channels,
                #      replicated across partitions ----
                def qs(q, t=None):
                    t = s_sb if t is None else t
                    return t[:, q * GRP:(q + 1) * GRP]

                SXX, SXY, SYY = qs(0), qs(1), qs(2)
                SR, SI = qs(3), qs(4)
                tmp = smallp.tile([P, 16 * GRP], f32, tag="tmp")

                def ts(i, tmp=tmp):
                    return tmp[:, i * GRP:(i + 1) * GRP]

                rN = 1.0 / N
                rN1 = 1.0 / (N - 1)
                MR, MI, u = ts(0), ts(1), ts(2)
                a, bb, cc = ts(3), ts(4), ts(5)
                V.tensor_scalar_mul(MR, SR, rN)
                V.tensor_scalar_mul(MI, SI, rN)
                # a=(Sxx-Sx*mr)/(N-1)+eps; b=(Sxy-Sx*mi)/(N-1);
                # c=(Syy-Sy*mi)/(N-1)+eps
                V.tensor_mul(u, SR, MR)
                V.tensor_sub(a, SXX, u)
                V.tensor_scalar(out=a, in0=a, scalar1=rN1, scalar2=EPS,
                                op0=Alu.mult, op1=Alu.add)
                V.tensor_mul(u, SR, MI)
                V.tensor_sub(bb, SXY, u)
                V.tensor_scalar_mul(bb, bb, rN1)
                V.tensor_mul(u, SI, MI)
                V.tensor_sub(cc, SYY, u)
                V.tensor_scalar(out=cc, in0=cc, scalar1=rN1, scalar2=EPS,
                                op0=Alu.mult, op1=Alu.add)
                # (M)^{-1/2} for M=[[a,b],[b,c]]: s=sqrt(ac-b^2);
                # t=sqrt(a+c+2s); W=[[c+s,-b],[-b,a+s]]/(s*t)
                det, s_, tr, st_, inv = ts(6), ts(7), ts(8), ts(9), ts(10)
                V.tensor_mul(det, a, cc)
                V.tensor_mul(u, bb, bb)
                V.tensor_sub(det, det, u)
                nc.scalar.sqrt(s_, det)
                V.tensor_add(u, a, cc)
                V.tensor_scalar_mul(tr, s_, 2.0)
                V.tensor_add(tr, tr, u)
                nc.scalar.sqrt(tr, tr)
                V.tensor_mul(st_, s_, tr)
                V.reciprocal(inv, st_)
                w00, w01, w11, q = ts(11), ts(12), ts(13), ts(14)
                V.tensor_add(w00, cc, s_)
                V.tensor_mul(w00, w00, inv)
                V.scalar_tensor_tensor(out=w01, in0=bb, scalar=-1.0, in1=inv,
                                       op0=Alu.mult, op1=Alu.mult)
                V.tensor_add(w11, a, s_)
                V.tensor_mul(w11, w11, inv)
                # G = gamma @ W ; B' = beta - G @ mean  (gb is quantity-major
                # over 8 channels; this group's 4 are contiguous)
                g00 = gb[:, 0 * 8 + grp * GRP: 0 * 8 + grp * GRP + GRP]
                g01 = gb[:, 1 * 8 + grp * GRP: 1 * 8 + grp * GRP + GRP]
                g10 = gb[:, 2 * 8 + grp * GRP: 2 * 8 + grp * GRP + GRP]
                g11 = gb[:, 3 * 8 + grp * GRP: 3 * 8 + grp * GRP + GRP]
                br_ = gb[:, 4 * 8 + grp * GRP: 4 * 8 + grp * GRP + GRP]
                bi_ = gb[:, 5 * 8 + grp * GRP: 5 * 8 + grp * GRP + GRP]
                cbt = smallp.tile([P, 6 * GRP], f32, tag="cb")
                G00, G01, BR = qs(0, cbt), qs(1, cbt), qs(2, cbt)
                G10, G11, BI = qs(3, cbt), qs(4, cbt), qs(5, cbt)
                V.tensor_mul(q, g00, w00)
                V.tensor_mul(u, g01, w01)
                V.tensor_add(G00, q, u)
                V.tensor_mul(q, g00, w01)
                V.tensor_mul(u, g01, w11)
                V.tensor_add(G01, q, u)
                V.tensor_mul(q, g10, w00)
                V.tensor_mul(u, g11, w01)
                V.tensor_add(G10, q, u)
                V.tensor_mul(q, g10, w01)
                V.tensor_mul(u, g11, w11)
                V.tensor_add(G11, q, u)
                q2 = ts(15)
                V.tensor_mul(q, G00, MR)
                V.tensor_mul(u, G01, MI)
                V.tensor_add(q2, q, u)
                V.tensor_sub(BR, br_, q2)
                V.tensor_mul(q, G10, MR)
                V.tensor_mul(u, G11, MI)
                V.tensor_add(q2, q, u)
                V.tensor_sub(BI, bi_, q2)

                # Bounce the coefficients through DRAM so the whiten ops
                # consume a DMA-produced tile (dependency-tracked path that
                # validated on hardware). Per-partition scalar operands are
                # exempt from the 2-byte packed-mode operand checks.
                nc.sync.dma_start(out=scratch_d[grp:grp + 1, :], in_=cbt[0:1, :])
                cbB = smallp.tile([P, 6 * GRP], f32, tag="cbB")
                nc.sync.dma_start(
                    out=cbB[:],
                    in_=scratch_d[grp:grp + 1, :].to_broadcast((P, 6 * GRP)))

                # ---- whiten + affine: ACT writes t=G*x+B straight into the
                #      output tile, DVE computes u=G'*xi (tensor_scalar, 4x
                #      eligible) and adds it in place (tensor_tensor, 2x
                #      eligible). All tensor operands flat packed bf16. ----
                for ci in range(GRP):
                    c = grp * GRP + ci
                    xt = xts[ci]
                    xr = xt[:, 0:F]
                    xi = xt[:, F:2 * F]

                    def cf(qi):
                        return cbB[:, qi * GRP + ci:qi * GRP + ci + 1]

                    yre = yp.tile([P, F], bf16, tag="y")
                    yim = yp.tile([P, F], bf16, tag="y")
                    u1 = upool.tile([P, F], bf16, tag="u")
                    u2 = upool.tile([P, F], bf16, tag="u")
                    # Both ACTs back-to-back (dense ScalarE queue), DVE's
                    # u-products run concurrently, then the two in-place
                    # adds, then the stores.
                    S.activation(yre[:], xr, Act.Identity,
                                 bias=cf(2), scale=cf(0))
                    S.activation(yim[:], xr, Act.Identity,
                                 bias=cf(5), scale=cf(3))
                    V.tensor_scalar_mul(u1[:], xi, cf(1))
                    V.tensor_scalar_mul(u2[:], xi, cf(4))
                    V.tensor_add(yre[:], yre[:], u1[:])
                    V.tensor_add(yim[:], yim[:], u2[:])
                    nc.sync.dma_start(out=y_d[c][:, 0:F], in_=yre[:])
                    nc.sync.dma_start(out=y_d[c][:, F:2 * F], in_=yim[:])

    nc.finalize()
    return nc


def _get_nc():
    if "nc" not in _CACHE:
        _CACHE["nc"] = _build_nc()
    return _CACHE["nc"]


def _prep_consts():
    return np.eye(P, dtype=np.float32)


def _prep_core(x_real, x_imag, gamma, beta, k):
    c0 = k * CLOC
    xy = np.empty((CLOC, P, XYW), BF16)
    xy[:, :, 0:F] = np.ascontiguousarray(
        x_real[:, c0:c0 + CLOC].transpose(1, 0, 2, 3)
    ).reshape(CLOC, P, F).astype(BF16)
    xy[:, :, F:2 * F] = np.ascontiguousarray(
        x_imag[:, c0:c0 + CLOC].transpose(1, 0, 2, 3)
    ).reshape(CLOC, P, F).astype(BF16)
    xy[:, :, 2 * F:] = 1.0
    g = gamma[c0:c0 + CLOC]
    b = beta[c0:c0 + CLOC]
    gb = np.concatenate([g[:, 0, 0], g[:, 0, 1], g[:, 1, 0], g[:, 1, 1],
                         b[:, 0], b[:, 1]]).astype(np.float32).reshape(1, 48)
    gb = np.broadcast_to(gb, (P, 48)).copy()
    return {"xy": xy, "consts": _prep_consts(), "gb": gb}


def kernel(x_real, x_imag, gamma, beta):
    from concourse.bass_utils import run_bass_kernel_spmd

    x_real = np.asarray(x_real, dtype=np.float32)
    x_imag = np.asarray(x_imag, dtype=np.float32)
    gamma = np.asarray(gamma, dtype=np.float32)
    beta = np.asarray(beta, dtype=np.float32)

    in_maps = [_prep_core(x_real, x_imag, gamma, beta, k)
               for k in range(NCORES)]

    nc = _get_nc()
    res = None
    if _TRACE:
        try:
            res = run_bass_kernel_spmd(nc, in_maps, list(range(NCORES)),
                                       trace=True)
        except Exception as e:  # trace infra unavailable -> plain run
            LAST["trace_error"] = repr(e)
            res = None
    if res is None:
        res = run_bass_kernel_spmd(nc, in_maps, list(range(NCORES)))
    LAST["exec_time_ns"] = res.exec_time_ns
    LAST["mean_exec_time_ns"] = res.mean_exec_time_ns
    LAST["profile_json"] = res.profile_json
    if res.instructions_and_trace is not None:
        LAST["trace_path"] = res.instructions_and_trace[1]

    out = np.empty((B, C, H, W, 2), np.float32)
    for k in range(NCORES):
        c0 = k * CLOC
        y = res.results[k]["y"]  # bf16 [CLOC, P, YW], planar [re(F)|im(F)]
        y = y.reshape(CLOC, P, 2, F).astype(np.float32)
        y = y.transpose(0, 2, 1, 3).reshape(CLOC, 2, B, H, W)
        out[:, c0:c0 + CLOC] = y.transpose(2, 0, 3, 4, 1)
    return out
